# revision 8
# baseline (speedup 1.0000x reference)
"""4D multilinear interpolation (8x8x8x8 lattice) on 8 Trainium2 cores.

For each row b: scale coordinates[b] (4 values in [0,1)) to the 7-cell
lattice, find the containing cell, gather the 16 corner values from
mesh_pred[b] (4096 values), and blend with multilinear weights.

Strategy (v2): replace per-partition indirect DMA (1 index/partition,
~1.4us Q7 descgen each, 32 needed) with 8 int16-indexed dma_gather
instructions (1024 descriptors each, CounterMachine-assisted descgen).
dma_gather addresses 256B-aligned blocks only, so each row gathers two
128-float spans at base = row*4096 + 512*ci0 + 64*ci1 (always 64-float
aligned); lattice dims 2,3 are folded into a 64-wide dot product with
the rank-1 hat-weight vector U2 (x) U3 built on DVE. The dot product
runs as one scalar_tensor_tensor multiply plus one mask-segmented
tensor_tensor_scan (state resets where mask==0), so the 64->1
reduction needs no add tree. Dims 0,1 use exact gather addressing and
a tiny 4-way weighted combine at the end.

Gather indices are host-precomputed int16 tables (DMA addressing
metadata); all value math stays on device in f32.
"""

import numpy as np

import concourse.bass as bass
import concourse.bacc as bacc
import concourse.mybir as mybir
from concourse import bass_utils, library_config
from concourse.tile import TileContext

P = 128            # partitions
N = 32             # row-columns per partition (P*N = 4096 rows/core)
BC = P * N         # rows per core
VOL = 4096         # 8^4 lattice values per row
NCORES = 8
GROUPS = 8         # dma_gather groups (int16 block-index range)
RPG = 512          # rows per group
NIDX = 1024        # gather slots per group (2 spans/row)
ELEM = 128         # f32 elements per gathered span (512B)
STEP = 64          # f32 element stride between indexable blocks (256B)
NROWS_AP = 32768   # indexable blocks per group window
GRP_ELS = RPG * VOL
PAD = 256          # AP window of last group reaches 64 els past the mesh
MESHN = BC * VOL + PAD

F32 = mybir.dt.float32
I16 = mybir.dt.int16
I32 = mybir.dt.int32
OP = mybir.AluOpType


def _build():
    nc = bacc.Bacc("TRN2", target_bir_lowering=False, debug=False)
    mesh = nc.dram_tensor("mesh", [MESHN], F32, kind="ExternalInput")
    cw = nc.dram_tensor("cw", [P, 136], F32, kind="ExternalInput")
    idx = nc.dram_tensor("idx16", [P, GROUPS * 64], I16, kind="ExternalInput")
    out = nc.dram_tensor("out", [P, N], F32, kind="ExternalOutput")

    mesh_t = mesh[:].tensor

    with TileContext(nc) as tc:
        with tc.tile_pool(name="pool", bufs=1) as pool:
            nc.gpsimd.load_library(library_config.mlp)

            idx_t = pool.tile([P, GROUPS * 64], I16, tag="idx")
            nc.sync.dma_start(out=idx_t[:], in_=idx[:])
            cw_t = pool.tile([P, 136], F32, tag="cw")
            nc.sync.dma_start(out=cw_t[:], in_=cw[:])

            def view(tile_ap, off, dims):
                return bass.AP(tile_ap.tensor, tile_ap.offset + off, [tile_ap.ap[0]] + dims)

            # ---- coordinate math (all f32) ----
            # call: [p, d*32 + n] scaled coords, d-major
            call = pool.tile([P, 128], F32, tag="call")
            nc.vector.tensor_scalar_mul(call[:], cw_t[:, 0:128], 7.0)
            # dims 0,1: floor via comparison sums (HW f32->i32 cast rounds,
            # CoreSim truncates -- compare-sums are exact on both)
            ges = []
            for k in range(1, 7):
                g = pool.tile([P, 64], F32, tag=f"ge{k}")
                nc.vector.tensor_scalar(out=g[:], in0=call[:, 0:64],
                                        scalar1=float(k), scalar2=None,
                                        op0=OP.is_ge)
                ges.append(g)
            while len(ges) > 1:
                nxt = []
                for i2 in range(0, len(ges) - 1, 2):
                    s2 = pool.tile([P, 64], F32, tag=f"gs{len(ges)}_{i2}")
                    nc.vector.tensor_tensor(out=s2[:], in0=ges[i2][:],
                                            in1=ges[i2 + 1][:], op=OP.add)
                    nxt.append(s2)
                if len(ges) % 2:
                    nxt.append(ges[-1])
                ges = nxt
            ci_f = ges[0]
            # WF: [p, s*64 + d*32 + n]; s=0 -> 1-frac, s=1 -> frac (d in {0,1})
            WF = pool.tile([P, 128], F32, tag="WF")
            nc.vector.tensor_tensor(out=WF[:, 64:128], in0=call[:, 0:64],
                                    in1=ci_f[:], op=OP.subtract)
            nc.vector.tensor_scalar(out=WF[:, 0:64], in0=WF[:, 64:128],
                                    scalar1=-1.0, scalar2=1.0,
                                    op0=OP.mult, op1=OP.add)

            # dims 2,3: hat weights U_d[q] = max(1 - |q - c_d|, 0), q in [0,8)
            # D23: [p, d*256 + n*8 + q] = iota_q - c_d  (d in {2,3})
            D23 = pool.tile([P, 512], F32, tag="D23")
            iota_v = view(cw_t[:], 128, [[0, 2], [0, 32], [1, 8]])
            c23_v = view(call[:], 64, [[32, 2], [1, 32], [0, 8]])
            nc.vector.tensor_tensor(out=D23[:], in0=iota_v, in1=c23_v, op=OP.subtract)
            # |.| and relu(1-|.|) on the otherwise-idle ACT engine
            bias1 = pool.tile([P, 1], F32, tag="bias1")
            nc.vector.memset(bias1[:], 1.0)
            N23 = pool.tile([P, 512], F32, tag="N23")
            nc.scalar.activation(out=N23[:], in_=D23[:],
                                 func=mybir.ActivationFunctionType.Abs)
            U23 = pool.tile([P, 512], F32, tag="U23")
            nc.scalar.activation(out=U23[:], in_=N23[:],
                                 func=mybir.ActivationFunctionType.Relu,
                                 bias=bias1[:, 0:1], scale=-1.0)
            # U64: [p, n*64 + Q*8 + R] = U2[n,Q] * U3[n,R]
            U64 = pool.tile([P, N * 64], F32, tag="U64")
            u2_v = view(U23[:], 0, [[8, 32], [1, 8], [0, 8]])
            u3_v = view(U23[:], 256, [[8, 32], [0, 8], [1, 8]])
            nc.vector.tensor_tensor(out=U64[:], in0=u2_v, in1=u3_v, op=OP.mult)

            # W01: [p, s*64 + b*32 + n] = w0[s,n] * w1[b,n]
            W01 = pool.tile([P, 128], F32, tag="W01")
            w0_v = view(WF[:], 0, [[64, 2], [0, 2], [1, 32]])
            w1_v = view(WF[:], 32, [[0, 2], [64, 2], [1, 32]])
            nc.vector.tensor_tensor(out=W01[:], in0=w0_v, in1=w1_v, op=OP.mult)

            # scan mask: 0 at each 64-el segment start, 1 elsewhere
            mask = pool.tile([P, 2048], F32, tag="mask")
            nc.vector.memset(mask[:], 1.0)
            mask0_v = view(mask[:], 0, [[64, 32], [1, 1]])
            nc.vector.memset(mask0_v, 0.0)

            # ---- gathers: 8 groups x 1024 slots x 128 floats ----
            Gbuf = pool.tile([P, GROUPS * NIDX], F32, tag="Gbuf")
            for g in range(GROUPS):
                out3 = Gbuf[:, NIDX * g:NIDX * (g + 1)].rearrange(
                    "p (k j) -> p k j", k=8)
                in_ap = bass.AP(mesh_t, g * GRP_ELS, [[STEP, NROWS_AP], [1, ELEM]])
                nc.gpsimd.dma_gather(
                    out_ap=out3,
                    in_ap=in_ap,
                    idxs_ap=idx_t[:, 64 * g:64 * (g + 1)],
                    num_idxs=NIDX,
                    num_idxs_reg=NIDX,
                    elem_size=ELEM,
                    elem_step=STEP,
                )

            # ---- blend: M = G * U64 (bcast), masked scan, tail combine ----
            S = pool.tile([P, GROUPS * NIDX], F32, tag="S")
            for c in range(4):
                Mc = pool.tile([P, 2048], F32, tag=f"M{c}")
                # 4 ops per chunk (one per (s,b) slot): 2-free-dim APs keep
                # InstTensorScalarPtr legal -> f32 2x_2p perf mode eligible
                for sb in range(4):
                    g_v = view(Gbuf[:], 2048 * c + 64 * sb, [[256, 8], [1, 64]])
                    m_v = view(Mc[:], 64 * sb, [[256, 8], [1, 64]])
                    u_v = view(U64[:], c * 512, [[64, 8], [1, 64]])
                    nc.vector.scalar_tensor_tensor(out=m_v, in0=g_v, scalar=1.0,
                                                   in1=u_v, op0=OP.mult,
                                                   op1=OP.mult)
                nc.vector.tensor_tensor_scan(
                    out=S[:, 2048 * c:2048 * (c + 1)], data0=mask[:], data1=Mc[:],
                    initial=0.0, op0=OP.mult, op1=OP.add)

            # E = segment sums at S[63::64]: order ((g,m), (s,b))
            T = pool.tile([P, 128], F32, tag="T")
            e_v = view(S[:], 63, [[256, 32], [64, 4]])
            w_v = view(W01[:], 0, [[1, 32], [32, 4]])
            nc.vector.scalar_tensor_tensor(out=T[:], in0=e_v, scalar=1.0,
                                           in1=w_v, op0=OP.mult, op1=OP.mult)
            T2 = pool.tile([P, 64], F32, tag="T2")
            nc.vector.tensor_tensor(out=T2[:], in0=T[:, 0::2], in1=T[:, 1::2],
                                    op=OP.add)
            acc = pool.tile([P, N], F32, tag="acc")
            nc.vector.tensor_tensor(out=acc[:], in0=T2[:, 0::2], in1=T2[:, 1::2],
                                    op=OP.add)
            nc.sync.dma_start(out=out[:], in_=acc[:])
    nc.compile()
    return nc


def _host_prep(coords_c, mesh_c):
    """Per-core input prep: d-major coords+iota, int16 gather tables, padded mesh."""
    c7 = coords_c.astype(np.float32) * np.float32(7.0)
    ci = c7.astype(np.int32)            # trunc == floor (c >= 0); 0..6
    ci0, ci1 = ci[:, 0], ci[:, 1]

    i = np.arange(NIDX)
    p, k = i % P, i // P
    m, s = k >> 1, k & 1
    lr = 128 * m + p
    idx16 = np.zeros((16, GROUPS * 64), np.int16)
    for g in range(GROUPS):
        r = RPG * g + lr
        vals = 64 * lr + 8 * (ci0[r] + s) + ci1[r]
        idx16[i % 16, g * 64 + i // 16] = vals.astype(np.int16)
    idx16 = np.tile(idx16, (8, 1))

    cwA = coords_c.reshape(N, P, 4).transpose(1, 2, 0).reshape(P, 128)
    iot = np.broadcast_to(np.arange(8, dtype=np.float32), (P, 8))
    cw = np.ascontiguousarray(np.concatenate([cwA, iot], axis=1), dtype=np.float32)

    mesh_flat = np.empty(MESHN, np.float32)
    mesh_flat[:BC * VOL] = mesh_c.reshape(-1)
    mesh_flat[BC * VOL:] = 0.0
    return {"mesh": mesh_flat, "cw": cw, "idx16": idx16}


_NC = None


def _get_nc():
    global _NC
    if _NC is None:
        _NC = _build()
    return _NC


def kernel(coordinates, mesh_pred, _trace=False, _tmpdir=None):
    coordinates = np.asarray(coordinates, dtype=np.float32)
    mesh_pred = np.asarray(mesh_pred, dtype=np.float32)
    assert coordinates.shape == (NCORES * BC, 4)
    assert mesh_pred.shape == (NCORES * BC, VOL)

    in_maps = []
    for cix in range(NCORES):
        sl = slice(cix * BC, (cix + 1) * BC)
        in_maps.append(_host_prep(coordinates[sl], mesh_pred[sl]))
    res = bass_utils.run_bass_kernel_spmd(
        _get_nc(),
        in_maps,
        core_ids=list(range(NCORES)),
        trace=_trace,
        tmpdir=_tmpdir,
    )
    outs = []
    for r in res.results:
        o = np.asarray(r["out"])              # [p, n]
        outs.append(o.transpose(1, 0).reshape(-1))  # row = n*P + p
    out = np.concatenate(outs)
    if _trace:
        return out, res
    return out


# revision 9
# speedup vs baseline: 1.4817x; 1.4817x over previous
"""4D multilinear interpolation (8x8x8x8 lattice) on 8 Trainium2 cores.

For each row b: scale coordinates[b] (4 values in [0,1)) to the 7-cell
lattice, find the containing cell, gather the 16 corner values from
mesh_pred[b] (4096 values), and blend with multilinear weights.

Strategy (v3): Q7 SWDGE descriptor generation is the bottleneck
(~8ns/descriptor for int16-indexed dma_gather, ~0.4us fixed per
instruction), so use exactly ONE descriptor per row: a 640-float span
at base = row*4096 + 512*ci0 + 64*ci1 (64-float aligned, int16 block
index) covers all 16 corners at offsets 512*a + 64*b + [0,64) for
a,b in {0,1}. 8 dma_gather instructions (512 rows each) move the
spans; DVE touches only the four needed 64-float windows per row via
strided views: one scalar_tensor_tensor multiply per (a,b) against
the rank-1 hat-weight vector U2 (x) U3 (folding lattice dims 2,3),
one segmented tensor_reduce per chunk for the 64->1 sums, then a tiny
w0 (x) w1 combine. Gather indices are host-precomputed int16 tables
(DMA addressing metadata); all value math stays on device in f32.
"""

import numpy as np

import concourse.bass as bass
import concourse.bacc as bacc
import concourse.mybir as mybir
from concourse import bass_utils, library_config
from concourse.tile import TileContext

P = 128            # partitions
N = 32             # row-columns per partition (P*N = 4096 rows/core)
BC = P * N         # rows per core
VOL = 4096         # 8^4 lattice values per row
NCORES = 8
GROUPS = 8         # dma_gather groups (int16 block-index range)
RPG = 512          # rows per group
NIDX = 512         # gather slots per group (1 span/row)
ELEM = 640         # f32 elements per gathered span (2560B)
STEP = 64          # f32 element stride between indexable blocks (256B)
NROWS_AP = 32768   # indexable blocks per group window
GRP_ELS = RPG * VOL
PAD = 1024         # AP window of last group reaches 576 els past the mesh
MESHN = BC * VOL + PAD

F32 = mybir.dt.float32
I16 = mybir.dt.int16
OP = mybir.AluOpType


def _build():
    nc = bacc.Bacc("TRN2", target_bir_lowering=False, debug=False)
    mesh = nc.dram_tensor("mesh", [MESHN], F32, kind="ExternalInput")
    cw = nc.dram_tensor("cw", [P, 136], F32, kind="ExternalInput")
    idx = nc.dram_tensor("idx16", [P, GROUPS * 32], I16, kind="ExternalInput")
    out = nc.dram_tensor("out", [P, N], F32, kind="ExternalOutput")

    mesh_t = mesh[:].tensor

    with TileContext(nc) as tc:
        with tc.tile_pool(name="pool", bufs=1) as pool:
            nc.gpsimd.load_library(library_config.mlp)

            idx_t = pool.tile([P, GROUPS * 32], I16, tag="idx")
            nc.sync.dma_start(out=idx_t[:], in_=idx[:])
            cw_t = pool.tile([P, 136], F32, tag="cw")
            nc.sync.dma_start(out=cw_t[:], in_=cw[:])

            def view(tile_ap, off, dims):
                return bass.AP(tile_ap.tensor, tile_ap.offset + off, [tile_ap.ap[0]] + dims)

            # ---- coordinate math (all f32) ----
            # call: [p, d*32 + n] scaled coords, d-major
            call = pool.tile([P, 128], F32, tag="call")
            nc.vector.tensor_scalar_mul(call[:], cw_t[:, 0:128], 7.0)
            # dims 0,1: floor via comparison sums (HW f32->i32 cast rounds,
            # CoreSim truncates -- compare-sums are exact on both)
            ges = []
            for k in range(1, 7):
                g = pool.tile([P, 64], F32, tag=f"ge{k}")
                nc.vector.tensor_scalar(out=g[:], in0=call[:, 0:64],
                                        scalar1=float(k), scalar2=None,
                                        op0=OP.is_ge)
                ges.append(g)
            while len(ges) > 1:
                nxt = []
                for i2 in range(0, len(ges) - 1, 2):
                    s2 = pool.tile([P, 64], F32, tag=f"gs{len(ges)}_{i2}")
                    nc.vector.tensor_tensor(out=s2[:], in0=ges[i2][:],
                                            in1=ges[i2 + 1][:], op=OP.add)
                    nxt.append(s2)
                if len(ges) % 2:
                    nxt.append(ges[-1])
                ges = nxt
            ci_f = ges[0]
            # WF: [p, s*64 + d*32 + n]; s=0 -> 1-frac, s=1 -> frac (d in {0,1})
            WF = pool.tile([P, 128], F32, tag="WF")
            nc.vector.tensor_tensor(out=WF[:, 64:128], in0=call[:, 0:64],
                                    in1=ci_f[:], op=OP.subtract)
            nc.vector.tensor_scalar(out=WF[:, 0:64], in0=WF[:, 64:128],
                                    scalar1=-1.0, scalar2=1.0,
                                    op0=OP.mult, op1=OP.add)

            # dims 2,3: hat weights U_d[q] = max(1 - |q - c_d|, 0), q in [0,8)
            # D23: [p, d*256 + n*8 + q] = iota_q - c_d  (d in {2,3})
            D23 = pool.tile([P, 512], F32, tag="D23")
            iota_v = view(cw_t[:], 128, [[0, 2], [0, 32], [1, 8]])
            c23_v = view(call[:], 64, [[32, 2], [1, 32], [0, 8]])
            nc.vector.tensor_tensor(out=D23[:], in0=iota_v, in1=c23_v, op=OP.subtract)
            # |.| and relu(1-|.|) on the otherwise-idle ACT engine
            bias1 = pool.tile([P, 1], F32, tag="bias1")
            nc.vector.memset(bias1[:], 1.0)
            N23 = pool.tile([P, 512], F32, tag="N23")
            nc.scalar.activation(out=N23[:], in_=D23[:],
                                 func=mybir.ActivationFunctionType.Abs)
            U23 = pool.tile([P, 512], F32, tag="U23")
            nc.scalar.activation(out=U23[:], in_=N23[:],
                                 func=mybir.ActivationFunctionType.Relu,
                                 bias=bias1[:, 0:1], scale=-1.0)
            # U64: [p, n*64 + Q*8 + R] = U2[n,Q] * U3[n,R]
            U64 = pool.tile([P, N * 64], F32, tag="U64")
            u2_v = view(U23[:], 0, [[8, 32], [1, 8], [0, 8]])
            u3_v = view(U23[:], 256, [[8, 32], [0, 8], [1, 8]])
            nc.vector.tensor_tensor(out=U64[:], in0=u2_v, in1=u3_v, op=OP.mult)

            # W01: [p, s*64 + b*32 + n] = w0[s,n] * w1[b,n]
            W01 = pool.tile([P, 128], F32, tag="W01")
            w0_v = view(WF[:], 0, [[64, 2], [0, 2], [1, 32]])
            w1_v = view(WF[:], 32, [[0, 2], [64, 2], [1, 32]])
            nc.vector.tensor_tensor(out=W01[:], in0=w0_v, in1=w1_v, op=OP.mult)

            # ---- gathers: 8 groups x 512 rows x 640-float span ----
            Gbuf = pool.tile([P, GROUPS * 4 * ELEM], F32, tag="Gbuf")
            for g in range(GROUPS):
                out3 = Gbuf[:, 4 * ELEM * g:4 * ELEM * (g + 1)].rearrange(
                    "p (k j) -> p k j", k=4)
                in_ap = bass.AP(mesh_t, g * GRP_ELS, [[STEP, NROWS_AP], [1, ELEM]])
                nc.gpsimd.dma_gather(
                    out_ap=out3,
                    in_ap=in_ap,
                    idxs_ap=idx_t[:, 32 * g:32 * (g + 1)],
                    num_idxs=NIDX,
                    num_idxs_reg=NIDX,
                    elem_size=ELEM,
                    elem_step=STEP,
                )

            # ---- blend: M = G * U64 on 4 windows/row, segmented reduce ----
            E = pool.tile([P, 128], F32, tag="E")
            for c in range(4):
                Mc = pool.tile([P, 2048], F32, tag=f"M{c}")
                # window (a,b) of row-slot: G offset 512a+64b, 64 els
                for ab in range(4):
                    a, b = ab >> 1, ab & 1
                    g_v = view(Gbuf[:], 2 * 4 * ELEM * c + 512 * a + 64 * b,
                               [[ELEM, 8], [1, 64]])
                    m_v = view(Mc[:], 64 * ab, [[256, 8], [1, 64]])
                    u_v = view(U64[:], c * 512, [[64, 8], [1, 64]])
                    nc.vector.scalar_tensor_tensor(out=m_v, in0=g_v, scalar=1.0,
                                                   in1=u_v, op0=OP.mult,
                                                   op1=OP.mult)
                mc3 = Mc[:].rearrange("p (s j) -> p s j", s=32)
                nc.vector.tensor_reduce(out=E[:, 32 * c:32 * (c + 1)], in_=mc3,
                                        axis=mybir.AxisListType.X, op=OP.add)

            # T = E * W01: order ((g,m), (a,b))
            T = pool.tile([P, 128], F32, tag="T")
            w_v = view(W01[:], 0, [[1, 32], [32, 4]])
            nc.vector.scalar_tensor_tensor(out=T[:], in0=E[:], scalar=1.0,
                                           in1=w_v, op0=OP.mult, op1=OP.mult)
            T2 = pool.tile([P, 64], F32, tag="T2")
            nc.vector.tensor_tensor(out=T2[:], in0=T[:, 0::2], in1=T[:, 1::2],
                                    op=OP.add)
            acc = pool.tile([P, N], F32, tag="acc")
            nc.vector.tensor_tensor(out=acc[:], in0=T2[:, 0::2], in1=T2[:, 1::2],
                                    op=OP.add)
            nc.sync.dma_start(out=out[:], in_=acc[:])
    nc.compile()
    return nc


def _host_prep(coords_c, mesh_c):
    """Per-core input prep: d-major coords+iota, int16 gather tables, padded mesh."""
    c7 = coords_c.astype(np.float32) * np.float32(7.0)
    ci = c7.astype(np.int32)            # trunc == floor (c >= 0); 0..6
    ci0, ci1 = ci[:, 0], ci[:, 1]

    i = np.arange(NIDX)
    p, m = i % P, i // P
    lr = 128 * m + p
    idx16 = np.zeros((16, GROUPS * 32), np.int16)
    for g in range(GROUPS):
        r = RPG * g + lr
        vals = 64 * lr + 8 * ci0[r] + ci1[r]
        idx16[i % 16, g * 32 + i // 16] = vals.astype(np.int16)
    idx16 = np.tile(idx16, (8, 1))

    cwA = coords_c.reshape(N, P, 4).transpose(1, 2, 0).reshape(P, 128)
    iot = np.broadcast_to(np.arange(8, dtype=np.float32), (P, 8))
    cw = np.ascontiguousarray(np.concatenate([cwA, iot], axis=1), dtype=np.float32)

    mesh_flat = np.empty(MESHN, np.float32)
    mesh_flat[:BC * VOL] = mesh_c.reshape(-1)
    mesh_flat[BC * VOL:] = 0.0
    return {"mesh": mesh_flat, "cw": cw, "idx16": idx16}


_NC = None


def _get_nc():
    global _NC
    if _NC is None:
        _NC = _build()
    return _NC


def kernel(coordinates, mesh_pred, _trace=False, _tmpdir=None):
    coordinates = np.asarray(coordinates, dtype=np.float32)
    mesh_pred = np.asarray(mesh_pred, dtype=np.float32)
    assert coordinates.shape == (NCORES * BC, 4)
    assert mesh_pred.shape == (NCORES * BC, VOL)

    in_maps = []
    for cix in range(NCORES):
        sl = slice(cix * BC, (cix + 1) * BC)
        in_maps.append(_host_prep(coordinates[sl], mesh_pred[sl]))
    res = bass_utils.run_bass_kernel_spmd(
        _get_nc(),
        in_maps,
        core_ids=list(range(NCORES)),
        trace=_trace,
        tmpdir=_tmpdir,
    )
    outs = []
    for r in res.results:
        o = np.asarray(r["out"])              # [p, n]
        outs.append(o.transpose(1, 0).reshape(-1))  # row = n*P + p
    out = np.concatenate(outs)
    if _trace:
        return out, res
    return out


# revision 10
# speedup vs baseline: 1.5135x; 1.0214x over previous
"""4D multilinear interpolation (8x8x8x8 lattice) on 8 Trainium2 cores.

For each row b: scale coordinates[b] (4 values in [0,1)) to the 7-cell
lattice, find the containing cell, gather the 16 corner values from
mesh_pred[b] (4096 values), and blend with multilinear weights.

Strategy (v4): Q7 SWDGE descriptor generation is the bottleneck
(~8ns/descriptor for int16-indexed dma_gather), so use exactly ONE
descriptor per row. The host relayouts each mesh row into 64
contiguous 256-float quads: quad beta = [block beta, beta+1, beta+8,
beta+9] (64-float blocks), an index-independent 4x duplication. One
1KB descriptor at beta = 8*ci0 + ci1 then fetches exactly the four
64-float windows holding the row's 16 corners: window (a,b) spans
lattice dims 2,3, which fold into a 64-wide dot with the rank-1
hat-weight vector U2 (x) U3 (one fused multiply per chunk + one
segmented tensor_reduce), then a tiny w0 (x) w1 combine. 8 dma_gather
instructions (512 rows each, int16 block indices) do all addressing;
indices/cell ids are host-precomputed metadata; all value math stays
on device in f32.
"""

import numpy as np

import concourse.bass as bass
import concourse.bacc as bacc
import concourse.mybir as mybir
from concourse import bass_utils, library_config
from concourse.tile import TileContext

P = 128            # partitions
N = 32             # row-columns per partition (P*N = 4096 rows/core)
BC = P * N         # rows per core
VOL = 4096         # 8^4 lattice values per row
NCORES = 8
GROUPS = 8         # dma_gather groups (int16 block-index range)
RPG = 512          # rows per group
NIDX = 512         # gather slots per group (1 quad/row)
ELEM = 256         # f32 elements per gathered quad (1KB)
STEP = 256         # f32 element stride between indexable quads
SLAB = 64 * ELEM   # relaid row size (16384 els)
NROWS_AP = 32768   # indexable quads per group window
GRP_ELS = RPG * SLAB
PAD = 1024
MESHN = BC * SLAB + PAD

F32 = mybir.dt.float32
I16 = mybir.dt.int16
OP = mybir.AluOpType


def _build():
    nc = bacc.Bacc("TRN2", target_bir_lowering=False, debug=False)
    mesh = nc.dram_tensor("mesh", [MESHN], F32, kind="ExternalInput")
    cw = nc.dram_tensor("cw", [P, 200], F32, kind="ExternalInput")
    idx = nc.dram_tensor("idx16", [P, GROUPS * 32], I16, kind="ExternalInput")
    out = nc.dram_tensor("out", [P, N], F32, kind="ExternalOutput")

    mesh_t = mesh[:].tensor

    with TileContext(nc) as tc:
        with tc.tile_pool(name="pool", bufs=1) as pool:
            nc.gpsimd.load_library(library_config.mlp)

            idx_t = pool.tile([P, GROUPS * 32], I16, tag="idx")
            nc.sync.dma_start(out=idx_t[:], in_=idx[:])
            cw_t = pool.tile([P, 200], F32, tag="cw")
            nc.sync.dma_start(out=cw_t[:], in_=cw[:])

            def view(tile_ap, off, dims):
                return bass.AP(tile_ap.tensor, tile_ap.offset + off, [tile_ap.ap[0]] + dims)

            # ---- coordinate math (all f32) ----
            # call: [p, d*32 + n] scaled coords, d-major
            call = pool.tile([P, 128], F32, tag="call")
            nc.vector.tensor_scalar_mul(call[:], cw_t[:, 0:128], 7.0)
            # WF: [p, s*64 + d*32 + n]; s=0 -> 1-frac, s=1 -> frac (d in {0,1})
            # cell ids ci01 for dims 0,1 arrive from the host (cols 128:192)
            WF = pool.tile([P, 128], F32, tag="WF")
            nc.vector.tensor_tensor(out=WF[:, 64:128], in0=call[:, 0:64],
                                    in1=cw_t[:, 128:192], op=OP.subtract)
            nc.vector.tensor_scalar(out=WF[:, 0:64], in0=WF[:, 64:128],
                                    scalar1=-1.0, scalar2=1.0,
                                    op0=OP.mult, op1=OP.add)

            # dims 2,3: hat weights U_d[q] = max(1 - |q - c_d|, 0), q in [0,8)
            # D23: [p, d*256 + n*8 + q] = iota_q - c_d  (d in {2,3})
            D23 = pool.tile([P, 512], F32, tag="D23")
            iota_v = view(cw_t[:], 192, [[0, 2], [0, 32], [1, 8]])
            c23_v = view(call[:], 64, [[32, 2], [1, 32], [0, 8]])
            nc.vector.tensor_tensor(out=D23[:], in0=iota_v, in1=c23_v, op=OP.subtract)
            # |.| and relu(1-|.|) on the otherwise-idle ACT engine
            bias1 = pool.tile([P, 1], F32, tag="bias1")
            nc.vector.memset(bias1[:], 1.0)
            N23 = pool.tile([P, 512], F32, tag="N23")
            nc.scalar.activation(out=N23[:], in_=D23[:],
                                 func=mybir.ActivationFunctionType.Abs)
            U23 = pool.tile([P, 512], F32, tag="U23")
            nc.scalar.activation(out=U23[:], in_=N23[:],
                                 func=mybir.ActivationFunctionType.Relu,
                                 bias=bias1[:, 0:1], scale=-1.0)
            # U64: [p, n*64 + Q*8 + R] = U2[n,Q] * U3[n,R]
            U64 = pool.tile([P, N * 64], F32, tag="U64")
            u2_v = view(U23[:], 0, [[8, 32], [1, 8], [0, 8]])
            u3_v = view(U23[:], 256, [[8, 32], [0, 8], [1, 8]])
            nc.vector.tensor_tensor(out=U64[:], in0=u2_v, in1=u3_v, op=OP.mult)

            # W01: [p, s*64 + b*32 + n] = w0[s,n] * w1[b,n]
            W01 = pool.tile([P, 128], F32, tag="W01")
            w0_v = view(WF[:], 0, [[64, 2], [0, 2], [1, 32]])
            w1_v = view(WF[:], 32, [[0, 2], [64, 2], [1, 32]])
            nc.vector.tensor_tensor(out=W01[:], in0=w0_v, in1=w1_v, op=OP.mult)

            # ---- gathers: 8 groups x 512 rows x 256-float quad ----
            Gbuf = pool.tile([P, GROUPS * 4 * ELEM], F32, tag="Gbuf")
            for g in range(GROUPS):
                out3 = Gbuf[:, 4 * ELEM * g:4 * ELEM * (g + 1)].rearrange(
                    "p (k j) -> p k j", k=4)
                in_ap = bass.AP(mesh_t, g * GRP_ELS, [[STEP, NROWS_AP], [1, ELEM]])
                nc.gpsimd.dma_gather(
                    out_ap=out3,
                    in_ap=in_ap,
                    idxs_ap=idx_t[:, 32 * g:32 * (g + 1)],
                    num_idxs=NIDX,
                    num_idxs_reg=NIDX,
                    elem_size=ELEM,
                    elem_step=STEP,
                )

            # ---- blend: one fused M = G * U64 per chunk + segmented reduce ----
            E = pool.tile([P, 128], F32, tag="E")
            for c in range(4):
                Mc = pool.tile([P, 2048], F32, tag=f"M{c}")
                g_v = Gbuf[:, 2048 * c:2048 * (c + 1)]
                u_v = view(U64[:], c * 512, [[64, 8], [0, 4], [1, 64]])
                nc.vector.tensor_tensor(out=Mc[:], in0=g_v, in1=u_v, op=OP.mult)
                mc3 = Mc[:].rearrange("p (s j) -> p s j", s=32)
                nc.vector.tensor_reduce(out=E[:, 32 * c:32 * (c + 1)], in_=mc3,
                                        axis=mybir.AxisListType.X, op=OP.add)

            # T = E * W01: order ((g,m), (a,b))
            T = pool.tile([P, 128], F32, tag="T")
            w_v = view(W01[:], 0, [[1, 32], [32, 4]])
            nc.vector.scalar_tensor_tensor(out=T[:], in0=E[:], scalar=1.0,
                                           in1=w_v, op0=OP.mult, op1=OP.mult)
            T2 = pool.tile([P, 64], F32, tag="T2")
            nc.vector.tensor_tensor(out=T2[:], in0=T[:, 0::2], in1=T[:, 1::2],
                                    op=OP.add)
            acc = pool.tile([P, N], F32, tag="acc")
            nc.vector.tensor_tensor(out=acc[:], in0=T2[:, 0::2], in1=T2[:, 1::2],
                                    op=OP.add)
            nc.sync.dma_start(out=out[:], in_=acc[:])
    nc.compile()
    return nc


def _host_prep(coords_c, mesh_c):
    """Per-core input prep: coords+cells+iota, int16 quad indices, quad-mesh."""
    c7 = coords_c.astype(np.float32) * np.float32(7.0)
    ci = c7.astype(np.int32)            # trunc == floor (c >= 0); 0..6
    ci0, ci1 = ci[:, 0], ci[:, 1]

    i = np.arange(NIDX)
    p, m = i % P, i // P
    lr = 128 * m + p
    idx16 = np.zeros((16, GROUPS * 32), np.int16)
    for g in range(GROUPS):
        r = RPG * g + lr
        vals = 64 * lr + 8 * ci0[r] + ci1[r]
        idx16[i % 16, g * 32 + i // 16] = vals.astype(np.int16)
    idx16 = np.tile(idx16, (8, 1))

    cwA = coords_c.reshape(N, P, 4).transpose(1, 2, 0).reshape(P, 128)
    ciA = ci[:, 0:2].astype(np.float32).reshape(N, P, 2).transpose(1, 2, 0).reshape(P, 64)
    iot = np.broadcast_to(np.arange(8, dtype=np.float32), (P, 8))
    cw = np.ascontiguousarray(np.concatenate([cwA, ciA, iot], axis=1),
                              dtype=np.float32)

    # quad relayout: row -> 64 quads of [block b, b+1, b+8, b+9] (64-el blocks)
    B = mesh_c.reshape(BC, 64, 64)
    mesh_flat = np.zeros(MESHN, np.float32)
    Q = mesh_flat[:BC * SLAB].reshape(BC, 64, 4, 64)
    Q[:, :55, 0] = B[:, 0:55]
    Q[:, :55, 1] = B[:, 1:56]
    Q[:, :55, 2] = B[:, 8:63]
    Q[:, :55, 3] = B[:, 9:64]
    return {"mesh": mesh_flat, "cw": cw, "idx16": idx16}


_NC = None


def _get_nc():
    global _NC
    if _NC is None:
        _NC = _build()
    return _NC


def kernel(coordinates, mesh_pred, _trace=False, _tmpdir=None):
    coordinates = np.asarray(coordinates, dtype=np.float32)
    mesh_pred = np.asarray(mesh_pred, dtype=np.float32)
    assert coordinates.shape == (NCORES * BC, 4)
    assert mesh_pred.shape == (NCORES * BC, VOL)

    in_maps = []
    for cix in range(NCORES):
        sl = slice(cix * BC, (cix + 1) * BC)
        in_maps.append(_host_prep(coordinates[sl], mesh_pred[sl]))
    res = bass_utils.run_bass_kernel_spmd(
        _get_nc(),
        in_maps,
        core_ids=list(range(NCORES)),
        trace=_trace,
        tmpdir=_tmpdir,
    )
    outs = []
    for r in res.results:
        o = np.asarray(r["out"])              # [p, n]
        outs.append(o.transpose(1, 0).reshape(-1))  # row = n*P + p
    out = np.concatenate(outs)
    if _trace:
        return out, res
    return out


# revision 11
# speedup vs baseline: 1.6518x; 1.0914x over previous
"""4D multilinear interpolation (8x8x8x8 lattice) on 8 Trainium2 cores.

For each row b: scale coordinates[b] (4 values in [0,1)) to the 7-cell
lattice, find the containing cell, gather the 16 corner values from
mesh_pred[b] (4096 values), and blend with multilinear weights.

Strategy (v4): Q7 SWDGE descriptor generation is the bottleneck
(~8ns/descriptor for int16-indexed dma_gather), so use exactly ONE
descriptor per row. The host relayouts each mesh row into 64
contiguous 256-float quads: quad beta = [block beta, beta+1, beta+8,
beta+9] (64-float blocks), an index-independent 4x duplication. One
1KB descriptor at beta = 8*ci0 + ci1 then fetches exactly the four
64-float windows holding the row's 16 corners: window (a,b) spans
lattice dims 2,3, which fold into a 64-wide dot with the rank-1
hat-weight vector U2 (x) U3 (one fused multiply per chunk + one
segmented tensor_reduce), then a tiny w0 (x) w1 combine. 8 dma_gather
instructions (512 rows each, int16 block indices) do all addressing;
indices/cell ids are host-precomputed metadata; all value math stays
on device in f32.
"""

import numpy as np

import concourse.bass as bass
import concourse.bacc as bacc
import concourse.mybir as mybir
from concourse import bass_utils, library_config
from concourse.tile import TileContext

P = 128            # partitions
N = 32             # row-columns per partition (P*N = 4096 rows/core)
BC = P * N         # rows per core
VOL = 4096         # 8^4 lattice values per row
NCORES = 8
GROUPS = 8         # dma_gather groups (int16 block-index range)
RPG = 512          # rows per group
NIDX = 512         # gather slots per group (1 quad/row)
ELEM = 256         # f32 elements per gathered quad (1KB)
STEP = 256         # f32 element stride between indexable quads
SLAB = 64 * ELEM   # relaid row size (16384 els)
NROWS_AP = 32768   # indexable quads per group window
GRP_ELS = RPG * SLAB
PAD = 1024
MESHN = BC * SLAB + PAD

F32 = mybir.dt.float32
I16 = mybir.dt.int16
OP = mybir.AluOpType


def _build():
    nc = bacc.Bacc("TRN2", target_bir_lowering=False, debug=False)
    mesh = nc.dram_tensor("mesh", [MESHN], F32, kind="ExternalInput")
    cw = nc.dram_tensor("cw", [P, 200], F32, kind="ExternalInput")
    idx = nc.dram_tensor("idx16", [P, GROUPS * 32], I16, kind="ExternalInput")
    out = nc.dram_tensor("out", [P, N], F32, kind="ExternalOutput")

    mesh_t = mesh[:].tensor

    with TileContext(nc) as tc:
        with tc.tile_pool(name="pool", bufs=1) as pool:
            nc.gpsimd.load_library(library_config.mlp)

            idx_t = pool.tile([P, GROUPS * 32], I16, tag="idx")
            nc.sync.dma_start(out=idx_t[:], in_=idx[:])
            cw_t = pool.tile([P, 200], F32, tag="cw")
            nc.sync.dma_start(out=cw_t[:], in_=cw[:])

            def view(tile_ap, off, dims):
                return bass.AP(tile_ap.tensor, tile_ap.offset + off, [tile_ap.ap[0]] + dims)

            # ---- coordinate math (all f32) ----
            # call: [p, d*32 + n] scaled coords, d-major
            call = pool.tile([P, 128], F32, tag="call")
            nc.vector.tensor_scalar_mul(call[:], cw_t[:, 0:128], 7.0)
            # WF: [p, s*64 + d*32 + n]; s=0 -> 1-frac, s=1 -> frac (d in {0,1})
            # cell ids ci01 for dims 0,1 arrive from the host (cols 128:192)
            WF = pool.tile([P, 128], F32, tag="WF")
            nc.vector.tensor_tensor(out=WF[:, 64:128], in0=call[:, 0:64],
                                    in1=cw_t[:, 128:192], op=OP.subtract)
            nc.vector.tensor_scalar(out=WF[:, 0:64], in0=WF[:, 64:128],
                                    scalar1=-1.0, scalar2=1.0,
                                    op0=OP.mult, op1=OP.add)

            # dims 2,3: hat weights U_d[q] = max(1 - |q - c_d|, 0), q in [0,8)
            # D23: [p, d*256 + n*8 + q] = iota_q - c_d  (d in {2,3})
            D23 = pool.tile([P, 512], F32, tag="D23")
            iota_v = view(cw_t[:], 192, [[0, 2], [0, 32], [1, 8]])
            c23_v = view(call[:], 64, [[32, 2], [1, 32], [0, 8]])
            nc.vector.tensor_tensor(out=D23[:], in0=iota_v, in1=c23_v, op=OP.subtract)
            # |.| and relu(1-|.|) on the otherwise-idle ACT engine
            bias1 = pool.tile([P, 1], F32, tag="bias1")
            nc.vector.memset(bias1[:], 1.0)
            N23 = pool.tile([P, 512], F32, tag="N23")
            nc.scalar.activation(out=N23[:], in_=D23[:],
                                 func=mybir.ActivationFunctionType.Abs)
            U23 = pool.tile([P, 512], F32, tag="U23")
            nc.scalar.activation(out=U23[:], in_=N23[:],
                                 func=mybir.ActivationFunctionType.Relu,
                                 bias=bias1[:, 0:1], scale=-1.0)
            # U64: [p, n*64 + Q*8 + R] = U2[n,Q] * U3[n,R]
            U64 = pool.tile([P, N * 64], F32, tag="U64")
            u2_v = view(U23[:], 0, [[8, 32], [1, 8], [0, 8]])
            u3_v = view(U23[:], 256, [[8, 32], [0, 8], [1, 8]])
            nc.vector.tensor_tensor(out=U64[:], in0=u2_v, in1=u3_v, op=OP.mult)

            # W01: [p, s*64 + b*32 + n] = w0[s,n] * w1[b,n]
            W01 = pool.tile([P, 128], F32, tag="W01")
            w0_v = view(WF[:], 0, [[64, 2], [0, 2], [1, 32]])
            w1_v = view(WF[:], 32, [[0, 2], [64, 2], [1, 32]])
            nc.vector.tensor_tensor(out=W01[:], in0=w0_v, in1=w1_v, op=OP.mult)

            # ---- gathers: 8 groups x 512 rows x 256-float quad ----
            Gbuf = pool.tile([P, GROUPS * 4 * ELEM], F32, tag="Gbuf")
            for g in range(GROUPS):
                out3 = Gbuf[:, 4 * ELEM * g:4 * ELEM * (g + 1)].rearrange(
                    "p (k j) -> p k j", k=4)
                in_ap = bass.AP(mesh_t, g * GRP_ELS, [[STEP, NROWS_AP], [1, ELEM]])
                nc.gpsimd.dma_gather(
                    out_ap=out3,
                    in_ap=in_ap,
                    idxs_ap=idx_t[:, 32 * g:32 * (g + 1)],
                    num_idxs=NIDX,
                    num_idxs_reg=NIDX,
                    elem_size=ELEM,
                    elem_step=STEP,
                )

            # ---- blend: one fused M = G * U64 per group + segmented reduce ----
            E = pool.tile([P, 128], F32, tag="E")
            for c in range(8):
                Mc = pool.tile([P, 1024], F32, tag=f"M{c}")
                g_v = Gbuf[:, 1024 * c:1024 * (c + 1)]
                u_v = view(U64[:], c * 256, [[64, 4], [0, 4], [1, 64]])
                nc.vector.tensor_tensor(out=Mc[:], in0=g_v, in1=u_v, op=OP.mult)
                mc3 = Mc[:].rearrange("p (s j) -> p s j", s=16)
                nc.vector.tensor_reduce(out=E[:, 16 * c:16 * (c + 1)], in_=mc3,
                                        axis=mybir.AxisListType.X, op=OP.add)

            # T = E * W01: order ((g,m), (a,b))
            T = pool.tile([P, 128], F32, tag="T")
            w_v = view(W01[:], 0, [[1, 32], [32, 4]])
            nc.vector.scalar_tensor_tensor(out=T[:], in0=E[:], scalar=1.0,
                                           in1=w_v, op0=OP.mult, op1=OP.mult)
            T2 = pool.tile([P, 64], F32, tag="T2")
            nc.vector.tensor_tensor(out=T2[:], in0=T[:, 0::2], in1=T[:, 1::2],
                                    op=OP.add)
            acc = pool.tile([P, N], F32, tag="acc")
            nc.vector.tensor_tensor(out=acc[:], in0=T2[:, 0::2], in1=T2[:, 1::2],
                                    op=OP.add)
            nc.sync.dma_start(out=out[:], in_=acc[:])
    nc.compile()
    return nc


def _host_prep(coords_c, mesh_c):
    """Per-core input prep: coords+cells+iota, int16 quad indices, quad-mesh."""
    c7 = coords_c.astype(np.float32) * np.float32(7.0)
    ci = c7.astype(np.int32)            # trunc == floor (c >= 0); 0..6
    ci0, ci1 = ci[:, 0], ci[:, 1]

    i = np.arange(NIDX)
    p, m = i % P, i // P
    lr = 128 * m + p
    idx16 = np.zeros((16, GROUPS * 32), np.int16)
    for g in range(GROUPS):
        r = RPG * g + lr
        vals = 64 * lr + 8 * ci0[r] + ci1[r]
        idx16[i % 16, g * 32 + i // 16] = vals.astype(np.int16)
    idx16 = np.tile(idx16, (8, 1))

    cwA = coords_c.reshape(N, P, 4).transpose(1, 2, 0).reshape(P, 128)
    ciA = ci[:, 0:2].astype(np.float32).reshape(N, P, 2).transpose(1, 2, 0).reshape(P, 64)
    iot = np.broadcast_to(np.arange(8, dtype=np.float32), (P, 8))
    cw = np.ascontiguousarray(np.concatenate([cwA, ciA, iot], axis=1),
                              dtype=np.float32)

    # quad relayout: row -> 64 quads of [block b, b+1, b+8, b+9] (64-el blocks)
    B = mesh_c.reshape(BC, 64, 64)
    mesh_flat = np.zeros(MESHN, np.float32)
    Q = mesh_flat[:BC * SLAB].reshape(BC, 64, 4, 64)
    Q[:, :55, 0] = B[:, 0:55]
    Q[:, :55, 1] = B[:, 1:56]
    Q[:, :55, 2] = B[:, 8:63]
    Q[:, :55, 3] = B[:, 9:64]
    return {"mesh": mesh_flat, "cw": cw, "idx16": idx16}


_NC = None


def _get_nc():
    global _NC
    if _NC is None:
        _NC = _build()
    return _NC


def kernel(coordinates, mesh_pred, _trace=False, _tmpdir=None):
    coordinates = np.asarray(coordinates, dtype=np.float32)
    mesh_pred = np.asarray(mesh_pred, dtype=np.float32)
    assert coordinates.shape == (NCORES * BC, 4)
    assert mesh_pred.shape == (NCORES * BC, VOL)

    in_maps = []
    for cix in range(NCORES):
        sl = slice(cix * BC, (cix + 1) * BC)
        in_maps.append(_host_prep(coordinates[sl], mesh_pred[sl]))
    res = bass_utils.run_bass_kernel_spmd(
        _get_nc(),
        in_maps,
        core_ids=list(range(NCORES)),
        trace=_trace,
        tmpdir=_tmpdir,
    )
    outs = []
    for r in res.results:
        o = np.asarray(r["out"])              # [p, n]
        outs.append(o.transpose(1, 0).reshape(-1))  # row = n*P + p
    out = np.concatenate(outs)
    if _trace:
        return out, res
    return out
